# revision 3
# baseline (speedup 1.0000x reference)
"""Trainium2 Bass kernel for nn_CombinedLoss (robot trajectory + phase loss).

Strategy: pure data-parallel over batch (32 batches/core x 8 cores), all
device traffic in bf16 (halves HBM bytes, unlocks DVE 2x/4x modes).  The
big sum-of-products reductions (MSE, vel cross-term, speed penalty,
phase x_gt, coherence numerator) run on the otherwise-idle PE via
PSUM-accumulated self-products + an identity-diagonal extract.  Host
reduces the per-core f32 partial strip in f64 and applies exact bf16
boundary corrections for cross-batch terms.
"""
import sys, os

for _p in (os.path.expanduser("~/.axon_site/_ro/trn_rl_repo"), "/opt/trn_rl_repo"):
    if os.path.isdir(_p) and _p not in sys.path:
        sys.path.insert(0, _p)

import numpy as np
import ml_dtypes
import concourse.bass as bass
import concourse.tile as tile
from concourse import bacc, mybir, bass_utils
from concourse.alu_op_type import AluOpType as OP

F32 = mybir.dt.float32
BF16 = mybir.dt.bfloat16
NPBF = ml_dtypes.bfloat16
AF = mybir.ActivationFunctionType
AX = mybir.AxisListType

# ---- problem constants (hardcoded) ----
B, T, D = 256, 8192, 12
NCORES = 8
BC = B // NCORES              # 32 batches per core
N = BC * T                    # 262144 frames per core
MAX_SPEED = 10.0

# robot chunking: 8 chunks x [128 partitions, 3072 cols]
W = 3072                      # elems per partition per chunk (256 frames)
NCH_R = (N * D) // (128 * W)  # 8
FR = W // D                   # 256 frames per partition per chunk
NBLK = W // 128               # 24 matmul blocks per chunk
SG = W // 3                   # 1024 speed-group cols per chunk

# phase chunking: 2 chunks x [128, 1024]
WP = 1024
NCH_P = N // (128 * WP)       # 2
WPE = WP + 1
PBLK = WP // 128              # 8

# strip columns
SV2 = 0                       # 8 cols: sum v^2 per robot chunk
SD2 = 8                       # sum (x-g)^2
SVV = 9                       # sum v_n * v_{n+1}
SPEN = 10                     # sum pen^2
SXG1 = 11
SXG2 = 12
SCO = 13
SCNT = 14                     # 2 cols
SX0 = 16
SLSE = 17
NCOLS = 18

# engine assignment knobs (tuned against TimelineSim)
CFG = {
    "d_mode": ["dve"] * NCH_R,            # dma | dve | pool
    "v_eng": ["dve"] * NCH_R,             # dve | pool
    "s2a_eng": ["dve"] * NCH_R,           # first group add
    "s2b_eng": ["pool"] * NCH_R,          # second group add
    "se_eng": ["pool"] * NCH_P,
    "m_eng": ["dve"] * NCH_P,
    "eq_eng": ["dve"] * NCH_P,
    "d10_eng": ["pool"] * NCH_P,
}


def _tt(nc, eng, out, a, b, op):
    (nc.gpsimd if eng == "pool" else nc.vector).tensor_tensor(out, a, b, op)


def build():
    nc = bacc.Bacc("TRN2", target_bir_lowering=False, debug=False)

    xr = nc.dram_tensor("xr", [N * D + 2 * D], BF16, kind="ExternalInput")
    gn = nc.dram_tensor("gn", [N * D], BF16, kind="ExternalInput")
    p0 = nc.dram_tensor("p0", [N + 2], BF16, kind="ExternalInput")
    p1 = nc.dram_tensor("p1", [N + 2], BF16, kind="ExternalInput")
    p2 = nc.dram_tensor("p2", [N + 2], BF16, kind="ExternalInput")
    gtf = nc.dram_tensor("gtf", [N], BF16, kind="ExternalInput")
    idt = nc.dram_tensor("idt", [128 * 128], BF16, kind="ExternalInput")
    out = nc.dram_tensor("partials", [128, NCOLS], F32, kind="ExternalOutput").ap()

    with tile.TileContext(nc) as tc:
        with tc.tile_pool(name="hold", bufs=1) as hold, \
             tc.tile_pool(name="ps", bufs=1, space="PSUM") as ps:
            strip = hold.tile([128, NCOLS], F32, name="strip")
            ident = hold.tile([128, 128], BF16, name="ident")
            nc.sync.dma_start(ident[:], bass.AP(idt, 0, [[128, 128], [1, 128]]))
            ones = hold.tile([128, 1], BF16, name="ones")
            nc.gpsimd.memset(ones[:], 1.0)
            s2hold = hold.tile([128, NCH_R * SG], BF16, name="s2hold")  # [128, 8192]

            bank_d2 = ps.tile([128, 512], F32, name="bank_d2")
            bank_vv = ps.tile([128, 512], F32, name="bank_vv")
            bank_pen = ps.tile([128, 512], F32, name="bank_pen")
            bank_xg1 = ps.tile([128, 512], F32, name="bank_xg1")
            bank_xg2 = ps.tile([128, 512], F32, name="bank_xg2")
            bank_co = ps.tile([128, 512], F32, name="bank_co")
            bank_x0 = ps.tile([128, 512], F32, name="bank_x0")
            bank_lse = ps.tile([128, 512], F32, name="bank_lse")

            # ---------------- phase pass (emitted first: exp/ln early) -------
            with tc.tile_pool(name="phase", bufs=2) as pp:
                for c in range(NCH_P):
                    base = c * 128 * WP
                    x0t = pp.tile([128, WPE], BF16, name="x0t")
                    x1t = pp.tile([128, WPE], BF16, name="x1t")
                    x2t = pp.tile([128, WPE], BF16, name="x2t")
                    gtt = pp.tile([128, WP], BF16, name="gtt")
                    nc.sync.dma_start(x0t[:], bass.AP(p0, base, [[WP, 128], [1, WPE]]))
                    nc.sync.dma_start(x1t[:], bass.AP(p1, base, [[WP, 128], [1, WPE]]))
                    nc.sync.dma_start(x2t[:], bass.AP(p2, base, [[WP, 128], [1, WPE]]))
                    nc.sync.dma_start(gtt[:], bass.AP(gtf, base, [[WP, 128], [1, WP]]))
                    # softmax denominator
                    e0 = pp.tile([128, WP], BF16, name="e0")
                    e1 = pp.tile([128, WP], BF16, name="e1")
                    e2 = pp.tile([128, WP], BF16, name="e2")
                    nc.scalar.activation(e0[:], x0t[:, 0:WP], AF.Exp)
                    nc.scalar.activation(e1[:], x1t[:, 0:WP], AF.Exp)
                    nc.scalar.activation(e2[:], x2t[:, 0:WP], AF.Exp)
                    se = pp.tile([128, WP], BF16, name="se")
                    eng = CFG["se_eng"][c]
                    _tt(nc, eng, se[:], e0[:], e1[:], OP.add)
                    _tt(nc, eng, se[:], se[:], e2[:], OP.add)
                    lnse = pp.tile([128, WP], BF16, name="lnse")
                    nc.scalar.activation(lnse[:], se[:], AF.Ln)
                    for k in range(PBLK):
                        nc.tensor.matmul(bank_lse[:, 0:1],
                                         lnse[:, k * 128:(k + 1) * 128], ones[:],
                                         start=(c == 0 and k == 0),
                                         stop=(c == NCH_P - 1 and k == PBLK - 1))
                    # argmax index via max + equality
                    m = pp.tile([128, WPE], BF16, name="m")
                    enm = CFG["m_eng"][c]
                    _tt(nc, enm, m[:], x0t[:], x1t[:], OP.max)
                    _tt(nc, enm, m[:], m[:], x2t[:], OP.max)
                    eq1 = pp.tile([128, WPE], BF16, name="eq1")
                    eq2 = pp.tile([128, WPE], BF16, name="eq2")
                    enq = CFG["eq_eng"][c]
                    _tt(nc, enq, eq1[:], x1t[:], m[:], OP.is_equal)
                    _tt(nc, enq, eq2[:], x2t[:], m[:], OP.is_equal)
                    idx = pp.tile([128, WPE], BF16, name="idx")
                    nc.vector.tensor_scalar(out=idx[:], in0=eq2[:], scalar1=2.0,
                                            scalar2=0.0, op0=OP.mult, op1=OP.add)
                    nc.vector.tensor_tensor(idx[:], idx[:], eq1[:], OP.add)
                    dd = pp.tile([128, WP], BF16, name="dd")
                    nc.vector.tensor_tensor(dd[:], idx[:, 1:WPE], idx[:, 0:WP],
                                            OP.subtract)
                    fmask = pp.tile([128, WP], BF16, name="fmask")
                    nc.vector.scalar_tensor_tensor(
                        out=fmask[:], in0=dd[:], scalar=-1.0, in1=dd[:],
                        op0=OP.add, op1=OP.mult)
                    mask = pp.tile([128, WP], BF16, name="mask")
                    nc.vector.tensor_scalar(out=mask[:], in0=fmask[:], scalar1=1.0,
                                            scalar2=0.0, op0=OP.min, op1=OP.add,
                                            accum_out=strip[:, SCNT + c:SCNT + c + 1])
                    msq = pp.tile([128, WP], BF16, name="msq")
                    nc.vector.tensor_tensor(msq[:], m[:, 1:WPE], m[:, 1:WPE], OP.mult)
                    for k in range(PBLK):
                        sl = slice(k * 128, (k + 1) * 128)
                        nc.tensor.matmul(bank_co[:, 0:128], mask[:, sl], msq[:, sl],
                                         start=(c == 0 and k == 0),
                                         stop=(c == NCH_P - 1 and k == PBLK - 1))
                    # x_gt pieces
                    d10 = pp.tile([128, WP], BF16, name="d10")
                    d21 = pp.tile([128, WP], BF16, name="d21")
                    engd = CFG["d10_eng"][c]
                    _tt(nc, engd, d10[:], x1t[:, 0:WP], x0t[:, 0:WP], OP.subtract)
                    _tt(nc, engd, d21[:], x2t[:, 0:WP], x1t[:, 0:WP], OP.subtract)
                    g1 = pp.tile([128, WP], BF16, name="g1")
                    g2 = pp.tile([128, WP], BF16, name="g2")
                    nc.vector.tensor_scalar(out=g1[:], in0=gtt[:], scalar1=1.0,
                                            scalar2=0.0, op0=OP.min, op1=OP.add)
                    nc.vector.tensor_scalar(out=g2[:], in0=gtt[:], scalar1=1.0,
                                            scalar2=0.0, op0=OP.subtract, op1=OP.max)
                    for k in range(PBLK):
                        sl = slice(k * 128, (k + 1) * 128)
                        nc.tensor.matmul(bank_xg1[:, 0:128], d10[:, sl], g1[:, sl],
                                         start=(c == 0 and k == 0),
                                         stop=(c == NCH_P - 1 and k == PBLK - 1))
                        nc.tensor.matmul(bank_xg2[:, 0:128], d21[:, sl], g2[:, sl],
                                         start=(c == 0 and k == 0),
                                         stop=(c == NCH_P - 1 and k == PBLK - 1))
                        nc.tensor.matmul(bank_x0[:, 0:1],
                                         x0t[:, k * 128:(k + 1) * 128], ones[:],
                                         start=(c == 0 and k == 0),
                                         stop=(c == NCH_P - 1 and k == PBLK - 1))

            # ---------------- robot pass ----------------
            with tc.tile_pool(name="robot", bufs=2) as rp, \
                 tc.tile_pool(name="spd", bufs=1) as spd:
                for c in range(NCH_R):
                    base = c * 128 * W
                    xt = rp.tile([128, W + 2 * D], BF16, name="xt")
                    nc.sync.dma_start(xt[:], bass.AP(xr, base, [[W, 128], [1, W + 2 * D]]))
                    # d = x - g
                    dmode = CFG["d_mode"][c]
                    if dmode == "dma":
                        dt_ = rp.tile([128, W], BF16, name="dt_")
                        nc.sync.dma_start(dt_[:], bass.AP(gn, base, [[W, 128], [1, W]]))
                        nc.gpsimd.dma_start(dt_[:], bass.AP(xr, base, [[W, 128], [1, W]]),
                                            accum_op=OP.add)
                    else:
                        gt_ = rp.tile([128, W], BF16, name="gt_")
                        nc.sync.dma_start(gt_[:], bass.AP(gn, base, [[W, 128], [1, W]]))
                        dt_ = rp.tile([128, W], BF16, name="dt_")
                        _tt(nc, dmode, dt_[:], xt[:, 0:W], gt_[:], OP.add)
                    # velocities (incl. one halo frame for the cross term)
                    v = rp.tile([128, W + D], BF16, name="v")
                    _tt(nc, CFG["v_eng"][c], v[:], xt[:, D:W + 2 * D], xt[:, 0:W + D],
                        OP.subtract)
                    # V2 = v^2 in SoA plane-major layout [128, 12*256]
                    V2 = rp.tile([128, W], BF16, name="V2")
                    pstr = V2[:].ap[0][0]
                    v2ap = bass.AP(V2.tensor, V2[:].offset, [[pstr, 128], [1, FR], [FR, D]])
                    nc.scalar.activation(v2ap, v[:, 0:W], AF.Square)
                    # s2 = per-(frame, group-of-3) sums from V2 planes
                    s2a = rp.tile([128, SG], BF16, name="s2a")
                    v2off = V2[:].offset
                    g0 = bass.AP(V2.tensor, v2off, [[pstr, 128], [3 * FR, 4], [1, FR]])
                    g1_ = bass.AP(V2.tensor, v2off + FR, [[pstr, 128], [3 * FR, 4], [1, FR]])
                    g2_ = bass.AP(V2.tensor, v2off + 2 * FR, [[pstr, 128], [3 * FR, 4], [1, FR]])
                    _tt(nc, CFG["s2a_eng"][c], s2a[:], g0, g1_, OP.add)
                    s2sl = s2hold[:, c * SG:(c + 1) * SG]
                    _tt(nc, CFG["s2b_eng"][c], s2sl, s2a[:], g2_, OP.add)
                    # sum v^2 for this chunk (from s2 groups)
                    jv = rp.tile([128, SG], BF16, name="jv", tag="jv")
                    nc.vector.tensor_scalar(out=jv[:], in0=s2sl, scalar1=1.0,
                                            scalar2=0.0, op0=OP.mult, op1=OP.add,
                                            accum_out=strip[:, SV2 + c:SV2 + c + 1])
                    # PE: sum d^2 and sum v_n v_{n+1}
                    for k in range(NBLK):
                        sl = slice(k * 128, (k + 1) * 128)
                        nc.tensor.matmul(bank_d2[:, 0:128], dt_[:, sl], dt_[:, sl],
                                         start=(c == 0 and k == 0),
                                         stop=(c == NCH_R - 1 and k == NBLK - 1))
                        slm = slice(D + k * 128, D + (k + 1) * 128)
                        nc.tensor.matmul(bank_vv[:, 0:128], v[:, sl], v[:, slm],
                                         start=(c == 0 and k == 0),
                                         stop=(c == NCH_R - 1 and k == NBLK - 1))

                    # speed burst over each completed half of s2hold
                    if c % 4 == 3:
                        h = c // 4
                        HW = NCH_R * SG // 2     # 4096
                        s2h = s2hold[:, h * HW:(h + 1) * HW]
                        r = spd.tile([128, HW], BF16, name="r", tag="r")
                        nc.scalar.activation(r[:], s2h, AF.Sqrt)
                        pen = spd.tile([128, HW], BF16, name="pen", tag="pen")
                        nc.vector.tensor_scalar(out=pen[:], in0=r[:], scalar1=MAX_SPEED,
                                                scalar2=0.0, op0=OP.subtract, op1=OP.max)
                        for k in range(HW // 128):
                            sl = slice(k * 128, (k + 1) * 128)
                            nc.tensor.matmul(bank_pen[:, 0:128], pen[:, sl], pen[:, sl],
                                             start=(h == 0 and k == 0),
                                             stop=(h == 1 and k == HW // 128 - 1))

            # ---------------- diag extracts + strip out ----------------
            with tc.tile_pool(name="fin", bufs=1) as fin:
                for bank, col in ((bank_d2, SD2), (bank_vv, SVV), (bank_pen, SPEN),
                                  (bank_xg1, SXG1), (bank_xg2, SXG2), (bank_co, SCO)):
                    jd = fin.tile([128, 128], F32, name="jd", tag="jd")
                    nc.vector.scalar_tensor_tensor(
                        out=jd[:], in0=bank[:, 0:128], scalar=0.0, in1=ident[:],
                        op0=OP.add, op1=OP.mult, accum_out=strip[:, col:col + 1])
                nc.vector.tensor_copy(strip[:, SX0:SX0 + 1], bank_x0[:, 0:1])
                nc.vector.tensor_copy(strip[:, SLSE:SLSE + 1], bank_lse[:, 0:1])
            nc.sync.dma_start(out, strip[:])
    nc.compile()
    return nc


_NC_CACHE = None


def _get_nc():
    global _NC_CACHE
    if _NC_CACHE is None:
        _NC_CACHE = build()
    return _NC_CACHE


_IDT = np.eye(128, dtype=NPBF).reshape(-1)


def _prep_core(xs, ps, gs, ts):
    """Per-core input map. xs,gs: [BC,T,D] f32; ps: [BC,T,3] f32; ts: [BC,T] i32."""
    xr = np.zeros((N * D + 2 * D,), NPBF)
    xr[:N * D] = xs.reshape(-1).astype(NPBF)
    gnv = (-gs.reshape(-1)).astype(NPBF)
    pf = ps.reshape(N, 3).astype(NPBF)
    pl = np.zeros((3, N + 2), NPBF)
    pl[:, :N] = pf.T
    return {
        "xr": xr,
        "gn": gnv,
        "p0": np.ascontiguousarray(pl[0]),
        "p1": np.ascontiguousarray(pl[1]),
        "p2": np.ascontiguousarray(pl[2]),
        "gtf": ts.astype(np.float32).astype(NPBF).reshape(-1),
        "idt": _IDT,
    }


def _host_finish(strips, pred_robot, pred_phase):
    """strips: list of [128, NCOLS] per core. Returns f32 scalar total loss."""
    S = np.stack([s.astype(np.float64).sum(axis=0) for s in strips])  # [8, NCOLS]
    tot = S.sum(axis=0)
    mse_sum = tot[SD2]
    svv = tot[SV2:SV2 + NCH_R].sum()
    scross = tot[SVV]
    sspeed = tot[SPEN]
    slse = tot[SLSE]
    sx0 = tot[SX0]
    sxg1 = tot[SXG1]
    sxg2 = tot[SXG2]
    scnt = tot[SCNT] + tot[SCNT + 1]
    sco = tot[SCO]

    # ---- boundary corrections (f64 from bf16-cast inputs, tiny) ----
    svv_c = 0.0; sspeed_c = 0.0; cross_c = 0.0; edge_sum = 0.0
    cnt_c = 0.0; co_c = 0.0
    xb_all = pred_robot.astype(NPBF).astype(np.float64)
    pb_all = pred_phase.astype(NPBF).astype(np.float64)
    for ci in range(NCORES):
        Xb = xb_all[ci * BC:(ci + 1) * BC]              # [BC,T,D]
        # invalid flat vels at n = k*T-1, k=1..BC
        vbad = np.empty((BC, D))
        vbad[:BC - 1] = Xb[1:, 0] - Xb[:-1, T - 1]
        vbad[BC - 1] = -Xb[BC - 1, T - 1]               # pad-zero edge
        svv_c += (vbad ** 2).sum()
        s2b = (vbad.reshape(BC, 4, 3) ** 2).sum(-1)
        pen = np.maximum(np.sqrt(s2b) - MAX_SPEED, 0.0)
        sspeed_c += (pen ** 2).sum()
        # invalid cross products: v_{nk-1}*vbad + vbad*v_{nk+1}
        vprev = Xb[:, T - 1] - Xb[:, T - 2]             # [BC,D] last valid vel
        vnext = Xb[:, 1] - Xb[:, 0]                     # first valid vel
        cross_c += (vprev * vbad).sum()
        cross_c += (vbad[:BC - 1] * vnext[1:]).sum()
        # per-batch edge vels for the acc identity
        edge_sum += (vnext ** 2).sum() + (vprev ** 2).sum()
        # phase coherence corrections at pair t = k*T-1
        Pb = pb_all[ci * BC:(ci + 1) * BC]              # [BC,T,3]
        a = Pb[:, T - 1]
        b = np.zeros_like(a)
        b[:BC - 1] = Pb[1:, 0]
        ma_ = a.max(-1); mb_ = b.max(-1)
        ua = (a[:, 1] == ma_) + 2.0 * (a[:, 2] == ma_)
        ub = (b[:, 1] == mb_) + 2.0 * (b[:, 2] == mb_)
        dd = ub - ua
        f = (dd - 1.0) * dd
        mask = np.minimum(f, 1.0)
        cnt_c += mask.sum()
        co_c += (mask * mb_ ** 2).sum()

    svv_t = svv - svv_c
    cross_t = scross - cross_c
    sspeed_t = sspeed - sspeed_c
    acc_sum = 2.0 * svv_t - edge_sum - 2.0 * cross_t
    cnt_t = scnt - cnt_c
    co_t = sco - co_c

    robot_loss = mse_sum / (B * T * D)
    xgt = sx0 + sxg1 + sxg2
    phase_loss = (slse - xgt) / (B * T)
    coherence = (co_t / max(cnt_t, 1.0)) if cnt_t > 0 else 0.0
    speed_loss = 5.0 * sspeed_t / (B * (T - 1) * 4)
    vel_loss = svv_t / (B * (T - 1) * D)
    acc_loss = acc_sum / (B * (T - 2) * D)
    total = (robot_loss + phase_loss + 10.0 * coherence + speed_loss
             + 0.05 * vel_loss + 0.01 * acc_loss)
    return np.asarray(total, dtype=np.float32)


def kernel(pred_robot, pred_phase, gt_robot, gt_phase):
    nc = _get_nc()
    in_maps = []
    for c in range(NCORES):
        sl = slice(c * BC, (c + 1) * BC)
        in_maps.append(_prep_core(pred_robot[sl], pred_phase[sl],
                                  gt_robot[sl], gt_phase[sl]))
    res = bass_utils.run_bass_kernel_spmd(nc, in_maps, core_ids=list(range(NCORES)))
    strips = [res.results[c]["partials"] for c in range(NCORES)]
    return _host_finish(strips, pred_robot, pred_phase)


# revision 4
# speedup vs baseline: 1.0693x; 1.0693x over previous
"""Trainium2 Bass kernel for nn_CombinedLoss (robot trajectory + phase loss).

Strategy: pure data-parallel over batch (32 batches/core x 8 cores), all
device traffic in bf16 (halves HBM bytes, unlocks DVE 2x/4x modes).  The
big sum-of-products reductions (MSE, vel cross-term, speed penalty,
phase x_gt, coherence numerator) run on the otherwise-idle PE via
PSUM-accumulated self-products + an identity-diagonal extract.  Host
reduces the per-core f32 partial strip in f64 and applies exact bf16
boundary corrections for cross-batch terms.
"""
import sys, os

for _p in (os.path.expanduser("~/.axon_site/_ro/trn_rl_repo"), "/opt/trn_rl_repo"):
    if os.path.isdir(_p) and _p not in sys.path:
        sys.path.insert(0, _p)

import numpy as np
import ml_dtypes
import concourse.bass as bass
import concourse.tile as tile
from concourse import bacc, mybir, bass_utils
from concourse.alu_op_type import AluOpType as OP

F32 = mybir.dt.float32
BF16 = mybir.dt.bfloat16
NPBF = ml_dtypes.bfloat16
AF = mybir.ActivationFunctionType
AX = mybir.AxisListType

# ---- problem constants (hardcoded) ----
B, T, D = 256, 8192, 12
NCORES = 8
BC = B // NCORES              # 32 batches per core
N = BC * T                    # 262144 frames per core
MAX_SPEED = 10.0

# robot chunking: 8 chunks x [128 partitions, 3072 cols]
W = 3072                      # elems per partition per chunk (256 frames)
NCH_R = (N * D) // (128 * W)  # 8
FR = W // D                   # 256 frames per partition per chunk
NBLK = W // 128               # 24 matmul blocks per chunk
SG = W // 3                   # 1024 speed-group cols per chunk

# phase chunking: 2 chunks x [128, 1024]
WP = 1024
NCH_P = N // (128 * WP)       # 2
WPE = WP + 1
PBLK = WP // 128              # 8

# strip columns
SV2 = 0                       # 8 cols: sum v^2 per robot chunk
SD2 = 8                       # sum (x-g)^2
SVV = 9                       # sum v_n * v_{n+1}
SPEN = 10                     # sum pen^2
SXG1 = 11
SXG2 = 12
SCO = 13
SCNT = 14                     # 2 cols
SX0 = 16
SLSE = 17
NCOLS = 18

# engine assignment knobs (tuned against TimelineSim)
CFG = {
    "d_mode": ["dve"] * NCH_R,            # dve | pool
    "v_eng": ["dve"] * NCH_R,             # dve | pool
    "s2a_eng": ["dve"] * NCH_R,           # first group add
    "s2b_eng": ["pool"] * NCH_R,          # second group add
    "se_eng": ["dve"] * NCH_P,
    "m_eng": ["dve"] * NCH_P,
    "eq_eng": ["dve"] * NCH_P,
    "d10_eng": ["pool"] * NCH_P,
}


def _tt(nc, eng, out, a, b, op):
    (nc.gpsimd if eng == "pool" else nc.vector).tensor_tensor(out, a, b, op)


def build():
    nc = bacc.Bacc("TRN2", target_bir_lowering=False, debug=False)

    xr = nc.dram_tensor("xr", [N * D + 2 * D], BF16, kind="ExternalInput")
    gn = nc.dram_tensor("gn", [N * D], BF16, kind="ExternalInput")
    p0 = nc.dram_tensor("p0", [N + 2], BF16, kind="ExternalInput")
    p1 = nc.dram_tensor("p1", [N + 2], BF16, kind="ExternalInput")
    p2 = nc.dram_tensor("p2", [N + 2], BF16, kind="ExternalInput")
    gtf = nc.dram_tensor("gtf", [N], BF16, kind="ExternalInput")
    idt = nc.dram_tensor("idt", [128 * 128], BF16, kind="ExternalInput")
    out = nc.dram_tensor("partials", [128, NCOLS], F32, kind="ExternalOutput").ap()

    with tile.TileContext(nc) as tc:
        with tc.tile_pool(name="hold", bufs=1) as hold, \
             tc.tile_pool(name="ps", bufs=1, space="PSUM") as ps:
            strip = hold.tile([128, NCOLS], F32, name="strip")
            ident = hold.tile([128, 128], BF16, name="ident")
            nc.sync.dma_start(ident[:], bass.AP(idt, 0, [[128, 128], [1, 128]]))
            ones = hold.tile([128, 1], BF16, name="ones")
            nc.gpsimd.memset(ones[:], 1.0)
            s2hold = hold.tile([128, NCH_R * SG], BF16, name="s2hold")  # [128, 8192]
            # persistent phase outputs consumed by deferred PE matmuls
            lnseH = hold.tile([128, NCH_P * WP], BF16, name="lnseH")
            maskH = hold.tile([128, NCH_P * WP], BF16, name="maskH")
            msqH = hold.tile([128, NCH_P * WP], BF16, name="msqH")
            d10H = hold.tile([128, NCH_P * WP], BF16, name="d10H")
            d21H = hold.tile([128, NCH_P * WP], BF16, name="d21H")
            g1H = hold.tile([128, NCH_P * WP], BF16, name="g1H")
            g2H = hold.tile([128, NCH_P * WP], BF16, name="g2H")
            x0H = hold.tile([128, NCH_P * WP], BF16, name="x0H")

            bank_d2 = ps.tile([128, 512], F32, name="bank_d2")
            bank_vv = ps.tile([128, 512], F32, name="bank_vv")
            bank_pen = ps.tile([128, 512], F32, name="bank_pen")
            bank_xg1 = ps.tile([128, 512], F32, name="bank_xg1")
            bank_xg2 = ps.tile([128, 512], F32, name="bank_xg2")
            bank_co = ps.tile([128, 512], F32, name="bank_co")
            bank_x0 = ps.tile([128, 512], F32, name="bank_x0")
            bank_lse = ps.tile([128, 512], F32, name="bank_lse")

            # ------------- phase compute (no matmuls; exp/ln early) -------------
            with tc.tile_pool(name="phase", bufs=2) as pp:
                for c in range(NCH_P):
                    base = c * 128 * WP
                    sl_ = slice(c * WP, (c + 1) * WP)
                    x0t = pp.tile([128, WPE], BF16, name="x0t")
                    x1t = pp.tile([128, WPE], BF16, name="x1t")
                    x2t = pp.tile([128, WPE], BF16, name="x2t")
                    gtt = pp.tile([128, WP], BF16, name="gtt")
                    nc.sync.dma_start(x0t[:], bass.AP(p0, base, [[WP, 128], [1, WPE]]))
                    nc.sync.dma_start(x1t[:], bass.AP(p1, base, [[WP, 128], [1, WPE]]))
                    nc.sync.dma_start(x2t[:], bass.AP(p2, base, [[WP, 128], [1, WPE]]))
                    nc.sync.dma_start(gtt[:], bass.AP(gtf, base, [[WP, 128], [1, WP]]))
                    nc.vector.tensor_copy(x0H[:, sl_], x0t[:, 0:WP])
                    # softmax denominator
                    e0 = pp.tile([128, WP], BF16, name="e0")
                    e1 = pp.tile([128, WP], BF16, name="e1")
                    e2 = pp.tile([128, WP], BF16, name="e2")
                    nc.scalar.activation(e0[:], x0t[:, 0:WP], AF.Exp)
                    nc.scalar.activation(e1[:], x1t[:, 0:WP], AF.Exp)
                    nc.scalar.activation(e2[:], x2t[:, 0:WP], AF.Exp)
                    se = pp.tile([128, WP], BF16, name="se")
                    eng = CFG["se_eng"][c]
                    _tt(nc, eng, se[:], e0[:], e1[:], OP.add)
                    _tt(nc, eng, se[:], se[:], e2[:], OP.add)
                    nc.scalar.activation(lnseH[:, sl_], se[:], AF.Ln)
                    # argmax index via max + equality
                    m = pp.tile([128, WPE], BF16, name="m")
                    enm = CFG["m_eng"][c]
                    _tt(nc, enm, m[:], x0t[:], x1t[:], OP.max)
                    _tt(nc, enm, m[:], m[:], x2t[:], OP.max)
                    eq1 = pp.tile([128, WPE], BF16, name="eq1")
                    eq2 = pp.tile([128, WPE], BF16, name="eq2")
                    enq = CFG["eq_eng"][c]
                    _tt(nc, enq, eq1[:], x1t[:], m[:], OP.is_equal)
                    _tt(nc, enq, eq2[:], x2t[:], m[:], OP.is_equal)
                    idx = pp.tile([128, WPE], BF16, name="idx")
                    nc.vector.tensor_scalar(out=idx[:], in0=eq2[:], scalar1=2.0,
                                            scalar2=0.0, op0=OP.mult, op1=OP.add)
                    nc.vector.tensor_tensor(idx[:], idx[:], eq1[:], OP.add)
                    dd = pp.tile([128, WP], BF16, name="dd")
                    nc.vector.tensor_tensor(dd[:], idx[:, 1:WPE], idx[:, 0:WP],
                                            OP.subtract)
                    # f = dd^2 - dd; mask = min(f, 1); count via accum
                    fmask = pp.tile([128, WP], BF16, name="fmask")
                    nc.vector.tensor_tensor(fmask[:], dd[:], dd[:], OP.mult)
                    nc.vector.tensor_tensor(fmask[:], fmask[:], dd[:], OP.subtract)
                    nc.vector.tensor_scalar(out=maskH[:, sl_], in0=fmask[:], scalar1=1.0,
                                            scalar2=0.0, op0=OP.min, op1=OP.add,
                                            accum_out=strip[:, SCNT + c:SCNT + c + 1])
                    nc.vector.tensor_tensor(msqH[:, sl_], m[:, 1:WPE], m[:, 1:WPE],
                                            OP.mult)
                    # x_gt pieces
                    engd = CFG["d10_eng"][c]
                    _tt(nc, engd, d10H[:, sl_], x1t[:, 0:WP], x0t[:, 0:WP], OP.subtract)
                    _tt(nc, engd, d21H[:, sl_], x2t[:, 0:WP], x1t[:, 0:WP], OP.subtract)
                    nc.vector.tensor_scalar(out=g1H[:, sl_], in0=gtt[:], scalar1=1.0,
                                            scalar2=0.0, op0=OP.min, op1=OP.add)
                    nc.vector.tensor_scalar(out=g2H[:, sl_], in0=gtt[:], scalar1=1.0,
                                            scalar2=0.0, op0=OP.subtract, op1=OP.max)

            def emit_phase_mms():
                NP_ = NCH_P * PBLK        # 16 blocks per quantity
                for k in range(NP_):
                    sl = slice(k * 128, (k + 1) * 128)
                    st = (k == 0)
                    sp = (k == NP_ - 1)
                    nc.tensor.matmul(bank_lse[:, 0:1], lnseH[:, sl], ones[:],
                                     start=st, stop=sp)
                    nc.tensor.matmul(bank_co[:, 0:128], maskH[:, sl], msqH[:, sl],
                                     start=st, stop=sp)
                    nc.tensor.matmul(bank_xg1[:, 0:128], d10H[:, sl], g1H[:, sl],
                                     start=st, stop=sp)
                    nc.tensor.matmul(bank_xg2[:, 0:128], d21H[:, sl], g2H[:, sl],
                                     start=st, stop=sp)
                    nc.tensor.matmul(bank_x0[:, 0:1], x0H[:, sl], ones[:],
                                     start=st, stop=sp)

            # ---------------- robot pass ----------------
            with tc.tile_pool(name="rdma", bufs=3) as rd, \
                 tc.tile_pool(name="rcmp", bufs=2) as rp, \
                 tc.tile_pool(name="spd", bufs=1) as spd:
                for c in range(NCH_R):
                    base = c * 128 * W
                    xt = rd.tile([128, W + 2 * D], BF16, name="xt")
                    nc.sync.dma_start(xt[:], bass.AP(xr, base, [[W, 128], [1, W + 2 * D]]))
                    gt_ = rd.tile([128, W], BF16, name="gt_")
                    nc.sync.dma_start(gt_[:], bass.AP(gn, base, [[W, 128], [1, W]]))
                    dt_ = rp.tile([128, W], BF16, name="dt_")
                    _tt(nc, CFG["d_mode"][c], dt_[:], xt[:, 0:W], gt_[:], OP.add)
                    # velocities (incl. one halo frame for the cross term)
                    v = rp.tile([128, W + D], BF16, name="v")
                    _tt(nc, CFG["v_eng"][c], v[:], xt[:, D:W + 2 * D], xt[:, 0:W + D],
                        OP.subtract)
                    # V2 = v^2 in SoA plane-major layout [128, 12*256]
                    V2 = rp.tile([128, W], BF16, name="V2")
                    pstr = V2[:].ap[0][0]
                    v2ap = bass.AP(V2.tensor, V2[:].offset, [[pstr, 128], [1, FR], [FR, D]])
                    nc.scalar.activation(v2ap, v[:, 0:W], AF.Square)
                    # s2 = per-(frame, group-of-3) sums from V2 planes
                    s2a = rp.tile([128, SG], BF16, name="s2a")
                    v2off = V2[:].offset
                    g0 = bass.AP(V2.tensor, v2off, [[pstr, 128], [3 * FR, 4], [1, FR]])
                    g1_ = bass.AP(V2.tensor, v2off + FR, [[pstr, 128], [3 * FR, 4], [1, FR]])
                    g2_ = bass.AP(V2.tensor, v2off + 2 * FR, [[pstr, 128], [3 * FR, 4], [1, FR]])
                    _tt(nc, CFG["s2a_eng"][c], s2a[:], g0, g1_, OP.add)
                    s2sl = s2hold[:, c * SG:(c + 1) * SG]
                    _tt(nc, CFG["s2b_eng"][c], s2sl, s2a[:], g2_, OP.add)
                    # sum v^2 for this chunk (from s2 groups)
                    jv = rp.tile([128, SG], BF16, name="jv", tag="jv")
                    nc.vector.tensor_scalar(out=jv[:], in0=s2sl, scalar1=1.0,
                                            scalar2=0.0, op0=OP.mult, op1=OP.add,
                                            accum_out=strip[:, SV2 + c:SV2 + c + 1])
                    # PE: sum d^2 and sum v_n v_{n+1}
                    for k in range(NBLK):
                        sl = slice(k * 128, (k + 1) * 128)
                        nc.tensor.matmul(bank_d2[:, 0:128], dt_[:, sl], dt_[:, sl],
                                         start=(c == 0 and k == 0),
                                         stop=(c == NCH_R - 1 and k == NBLK - 1))
                        slm = slice(D + k * 128, D + (k + 1) * 128)
                        nc.tensor.matmul(bank_vv[:, 0:128], v[:, sl], v[:, slm],
                                         start=(c == 0 and k == 0),
                                         stop=(c == NCH_R - 1 and k == NBLK - 1))

                    if c == 2:
                        emit_phase_mms()

                    # speed burst over each completed half of s2hold
                    if c % 4 == 3:
                        h = c // 4
                        HW = NCH_R * SG // 2     # 4096
                        s2h = s2hold[:, h * HW:(h + 1) * HW]
                        r = spd.tile([128, HW], BF16, name="r", tag="r")
                        nc.scalar.activation(r[:], s2h, AF.Sqrt)
                        pen = spd.tile([128, HW], BF16, name="pen", tag="pen")
                        nc.vector.tensor_scalar(out=pen[:], in0=r[:], scalar1=MAX_SPEED,
                                                scalar2=0.0, op0=OP.subtract, op1=OP.max)
                        for k in range(HW // 128):
                            sl = slice(k * 128, (k + 1) * 128)
                            nc.tensor.matmul(bank_pen[:, 0:128], pen[:, sl], pen[:, sl],
                                             start=(h == 0 and k == 0),
                                             stop=(h == 1 and k == HW // 128 - 1))

            # ---------------- diag extracts + strip out ----------------
            with tc.tile_pool(name="fin", bufs=1) as fin:
                for bank, col in ((bank_d2, SD2), (bank_vv, SVV), (bank_pen, SPEN),
                                  (bank_xg1, SXG1), (bank_xg2, SXG2), (bank_co, SCO)):
                    jd = fin.tile([128, 128], F32, name="jd", tag="jd")
                    nc.vector.scalar_tensor_tensor(
                        out=jd[:], in0=bank[:, 0:128], scalar=0.0, in1=ident[:],
                        op0=OP.add, op1=OP.mult, accum_out=strip[:, col:col + 1])
                nc.vector.tensor_copy(strip[:, SX0:SX0 + 1], bank_x0[:, 0:1])
                nc.vector.tensor_copy(strip[:, SLSE:SLSE + 1], bank_lse[:, 0:1])
            nc.sync.dma_start(out, strip[:])
    nc.compile()
    return nc


_NC_CACHE = None


def _get_nc():
    global _NC_CACHE
    if _NC_CACHE is None:
        _NC_CACHE = build()
    return _NC_CACHE


_IDT = np.eye(128, dtype=NPBF).reshape(-1)


def _prep_core(xs, ps, gs, ts):
    """Per-core input map. xs,gs: [BC,T,D] f32; ps: [BC,T,3] f32; ts: [BC,T] i32."""
    xr = np.zeros((N * D + 2 * D,), NPBF)
    xr[:N * D] = xs.reshape(-1).astype(NPBF)
    gnv = (-gs.reshape(-1)).astype(NPBF)
    pf = ps.reshape(N, 3).astype(NPBF)
    pl = np.zeros((3, N + 2), NPBF)
    pl[:, :N] = pf.T
    return {
        "xr": xr,
        "gn": gnv,
        "p0": np.ascontiguousarray(pl[0]),
        "p1": np.ascontiguousarray(pl[1]),
        "p2": np.ascontiguousarray(pl[2]),
        "gtf": ts.astype(np.float32).astype(NPBF).reshape(-1),
        "idt": _IDT,
    }


def _host_finish(strips, pred_robot, pred_phase):
    """strips: list of [128, NCOLS] per core. Returns f32 scalar total loss."""
    S = np.stack([s.astype(np.float64).sum(axis=0) for s in strips])  # [8, NCOLS]
    tot = S.sum(axis=0)
    mse_sum = tot[SD2]
    svv = tot[SV2:SV2 + NCH_R].sum()
    scross = tot[SVV]
    sspeed = tot[SPEN]
    slse = tot[SLSE]
    sx0 = tot[SX0]
    sxg1 = tot[SXG1]
    sxg2 = tot[SXG2]
    scnt = tot[SCNT] + tot[SCNT + 1]
    sco = tot[SCO]

    # ---- boundary corrections (f64 from bf16-cast inputs, tiny) ----
    svv_c = 0.0; sspeed_c = 0.0; cross_c = 0.0; edge_sum = 0.0
    cnt_c = 0.0; co_c = 0.0
    xb_all = pred_robot.astype(NPBF).astype(np.float64)
    pb_all = pred_phase.astype(NPBF).astype(np.float64)
    for ci in range(NCORES):
        Xb = xb_all[ci * BC:(ci + 1) * BC]              # [BC,T,D]
        # invalid flat vels at n = k*T-1, k=1..BC
        vbad = np.empty((BC, D))
        vbad[:BC - 1] = Xb[1:, 0] - Xb[:-1, T - 1]
        vbad[BC - 1] = -Xb[BC - 1, T - 1]               # pad-zero edge
        svv_c += (vbad ** 2).sum()
        s2b = (vbad.reshape(BC, 4, 3) ** 2).sum(-1)
        pen = np.maximum(np.sqrt(s2b) - MAX_SPEED, 0.0)
        sspeed_c += (pen ** 2).sum()
        # invalid cross products: v_{nk-1}*vbad + vbad*v_{nk+1}
        vprev = Xb[:, T - 1] - Xb[:, T - 2]             # [BC,D] last valid vel
        vnext = Xb[:, 1] - Xb[:, 0]                     # first valid vel
        cross_c += (vprev * vbad).sum()
        cross_c += (vbad[:BC - 1] * vnext[1:]).sum()
        # per-batch edge vels for the acc identity
        edge_sum += (vnext ** 2).sum() + (vprev ** 2).sum()
        # phase coherence corrections at pair t = k*T-1
        Pb = pb_all[ci * BC:(ci + 1) * BC]              # [BC,T,3]
        a = Pb[:, T - 1]
        b = np.zeros_like(a)
        b[:BC - 1] = Pb[1:, 0]
        ma_ = a.max(-1); mb_ = b.max(-1)
        ua = (a[:, 1] == ma_) + 2.0 * (a[:, 2] == ma_)
        ub = (b[:, 1] == mb_) + 2.0 * (b[:, 2] == mb_)
        dd = ub - ua
        f = (dd - 1.0) * dd
        mask = np.minimum(f, 1.0)
        cnt_c += mask.sum()
        co_c += (mask * mb_ ** 2).sum()

    svv_t = svv - svv_c
    cross_t = scross - cross_c
    sspeed_t = sspeed - sspeed_c
    acc_sum = 2.0 * svv_t - edge_sum - 2.0 * cross_t
    cnt_t = scnt - cnt_c
    co_t = sco - co_c

    robot_loss = mse_sum / (B * T * D)
    xgt = sx0 + sxg1 + sxg2
    phase_loss = (slse - xgt) / (B * T)
    coherence = (co_t / max(cnt_t, 1.0)) if cnt_t > 0 else 0.0
    speed_loss = 5.0 * sspeed_t / (B * (T - 1) * 4)
    vel_loss = svv_t / (B * (T - 1) * D)
    acc_loss = acc_sum / (B * (T - 2) * D)
    total = (robot_loss + phase_loss + 10.0 * coherence + speed_loss
             + 0.05 * vel_loss + 0.01 * acc_loss)
    return np.asarray(total, dtype=np.float32)


def kernel(pred_robot, pred_phase, gt_robot, gt_phase):
    nc = _get_nc()
    in_maps = []
    for c in range(NCORES):
        sl = slice(c * BC, (c + 1) * BC)
        in_maps.append(_prep_core(pred_robot[sl], pred_phase[sl],
                                  gt_robot[sl], gt_phase[sl]))
    res = bass_utils.run_bass_kernel_spmd(nc, in_maps, core_ids=list(range(NCORES)))
    strips = [res.results[c]["partials"] for c in range(NCORES)]
    return _host_finish(strips, pred_robot, pred_phase)
